# revision 40
# baseline (speedup 1.0000x reference)
"""Trainium2 Bass kernel for nn_EpipolarLoss (epipolar angular loss).

Reference semantics (per batch row b):
    dp     = sum_c(y_hat_c * y_c) / ||y_hat||          [per pixel]
    theta  = acos(clip(dp, -1+eps, 1-eps))
    vals   = sort(theta_row)
    cond_b = vals[pid] < 0.1            (pid = N//2)
    loss_b = sum(pid smallest)          if cond_b
           = sum(theta[theta < 0.1])    otherwise
    loss   = sum_b loss_b

Device strategy (pure data parallel, B=32 over 8 cores, 4 rows/core):
  The sort is avoided: cond_b <=> count(theta < 0.1) > pid.  Each core
  computes, per processing chunk, per SBUF partition, SUMP = sum over
  masked pixels of theta (mask: theta < 0.1).  For masked pixels theta is
  evaluated with the small-angle series acos(x) ~ sqrt(2u), u = 1-x
  (|relative error| <= ~5e-7 on the masked domain), so no acos LUT is
  needed.  Engine split per chunk (ScalarE's Rsqrt/Reciprocal LUTs are
  banned for accuracy, and everything here fits one activation table set,
  natural_log_exp_and_others):
    DVE : prod = yh*y;  paired channel adds -> (dot, n2);  w = dot*rsq2;
          fused masked sum:  mth = (relu_out < mthresh)*s  [+accum]
    ACT : sq = yh^2;  rsq2 = exp(-0.5*ln(n2) + ln2) = 2/sqrt(n2);
          relu_out = relu((2-umin) - w)  [= clamp of u3 = 2-2*dp];
          s = exp(0.5*ln(relu_out + umin)) = sqrt(u3) ~= theta
  GpSimd/PE are intentionally idle: GpSimd shares an SBUF port with DVE
  (measured contention), PE fp32 matmul is 4 cyc/col and fp32r is a
  reduced-precision format.
  The host combines chunk partials, proves cond_b false via
  count <= SUMP/theta_min (theta_min = acos(1-1e-7) ~ 4.88e-4), and falls
  back to an exact numpy sort mirror of the reference for any ambiguous
  row (never taken for randn-scale inputs; also catches NaNs).
"""

import numpy as np

# ---------------- problem constants (hardcoded, self-contained) -------------
B, C, H, W = 32, 3, 384, 512
N = H * W                      # 196608 pixels per row
PID = int(N * 0.5)             # 98304
THRESH = 0.1
EPS = 1e-7
NCORES = 8
ROWS_PER_CORE = B // NCORES    # 4
P = 128                        # SBUF partitions
F = N // P                     # 1536 pixels per partition per row
# processing chunks (row, j0, j1) in per-partition pixel units: row 0 split in
# quarters (fast pipeline ramp-in), row 3 in halves (fast drain)
CHUNKS = [(0, 0, 384), (0, 384, 768), (0, 768, 1152), (0, 1152, 1536),
          (1, 0, 512), (1, 512, 1024), (1, 1024, 1536),
          (2, 0, 512), (2, 512, 1024), (2, 1024, 1536),
          (3, 0, 768), (3, 768, 1536)]

_f32 = np.float32
UMIN = float(_f32(2.0) - _f32(2.0) * (_f32(1.0) - _f32(EPS)))   # 2-2*(1-eps), exact
C01 = float(_f32(np.cos(THRESH)))                               # cos(0.1) in fp32
UMASK = float(_f32(2.0) - _f32(2.0) * _f32(C01))
MTHRESH = float(_f32(UMASK) - _f32(UMIN))        # mask: relu_out < MTHRESH
RELU_BIAS = float(_f32(2.0) - _f32(UMIN))        # relu(u + (2-umin))
P_SCALE = 1.0 / 24.0                             # p = 1 + u3/24  (u3 = 2u)
P_BIAS = float(_f32(1.0) + _f32(UMIN / 24.0))
# p upper bound on masked pixels (for exact-count recovery bounds)
P_MAX = 1.0 + UMASK / 24.0 + 1e-6

# count ambiguity slack (mask boundary flips are ~O(1) pixels; be generous)
CNT_SLACK = 4096.0

# perf/accuracy knobs
PRECISE_P = False   # True: keep p=1+u3/24 factor + p-weighted count (extra DVE op)
PROD_ON_POOL = False  # GpSimd shares an SBUF port with DVE: contention loses
W_ON_POOL = False  # Pool w-mult measured worse (port contention + chain latency)
LN2 = float(np.log(2.0))

# ---------------- device kernel ---------------------------------------------
_CACHE = {}


def _patch_act_tables():
    """Force every activation onto the natural_log_exp_and_others table set.

    bacc's set chooser otherwise flip-flops between sets (Ln -> natural_log,
    Exp -> exp_and_others, ...) costing ~1.3us per ACT_TABLE_LOAD.  All
    functions used here (Square/Ln/Exp/Relu/Sign/Copy) live in one set, so
    blank out every other set (keeping dict order => act_func_set_id stays
    aligned with act_info.json).
    """
    import concourse.bacc as bacc_mod
    import concourse.hw_specs as hw_specs

    if getattr(bacc_mod, "_epi_act_patch", False):
        return
    orig = hw_specs.get_activation_tables

    def patched(arch):
        t = orig(arch)
        keep = "natural_log_exp_and_others"
        assert keep in t
        return {k: (v if k == keep else set()) for k, v in t.items()}

    bacc_mod.get_activation_tables = patched
    bacc_mod._epi_act_patch = True


def _build_nc():
    import concourse.bass as bass  # noqa: F401  (AP types)
    import concourse.tile as tile
    from concourse import bacc, mybir

    _patch_act_tables()

    F32 = mybir.dt.float32
    ALU = mybir.AluOpType
    ACT = mybir.ActivationFunctionType

    nc = bacc.Bacc(dynamic_dma_scratch_size=4096)
    yh_d = nc.dram_tensor("yh", [ROWS_PER_CORE, C, H, W], F32, kind="ExternalInput")
    yy_d = nc.dram_tensor("yy", [ROWS_PER_CORE, C, H, W], F32, kind="ExternalInput")
    out_d = nc.dram_tensor("partials", [P, 2 * len(CHUNKS)], F32,
                           kind="ExternalOutput")

    with tile.TileContext(nc, pool_alloc_mode="queue") as tc:
        with (
            tc.tile_pool(name="pin", bufs=2) as pin,
            tc.tile_pool(name="pbig", bufs=1) as pbig,
            tc.tile_pool(name="psm", bufs=1) as psm,
            tc.tile_pool(name="pconst", bufs=1) as pconst,
        ):
            cb_relu = pconst.tile([P, 1], F32, name="cb_relu", tag="cb_relu")
            nc.vector.memset(cb_relu[:, :], RELU_BIAS)
            cb_umin = pconst.tile([P, 1], F32, name="cb_umin", tag="cb_umin")
            nc.vector.memset(cb_umin[:, :], UMIN)
            cb_mth = pconst.tile([P, 1], F32, name="cb_mth", tag="cb_mth")
            nc.vector.memset(cb_mth[:, :], MTHRESH)
            cb_ln2 = pconst.tile([P, 1], F32, name="cb_ln2", tag="cb_ln2")
            nc.vector.memset(cb_ln2[:, :], LN2)
            cb_zero = pconst.tile([P, 1], F32, name="cb_zero", tag="cb_zero")
            nc.vector.memset(cb_zero[:, :], 0.0)
            out_t = pconst.tile([P, 2 * len(CHUNKS)], F32, name="out_t",
                                tag="out_t")

            # software-pipelined mth: chunk ci's masked-sum STT is emitted
            # during chunk ci+1 (right after its t2 add), so the DVE never
            # stalls waiting for the 3-op ACT chain that produces s_ci.
            pending = []  # (relu_t, s_t, ci)

            def flush_mth():
                while pending:
                    prelu, ps_t, pci = pending.pop(0)
                    mth = psm.tile([P, prelu.shape[-1]], F32, name="mth",
                                   tag="mth", bufs=2)
                    nc.vector.scalar_tensor_tensor(
                        mth[:, :], prelu[:, :], MTHRESH, ps_t[:, :],
                        ALU.is_lt, ALU.mult,
                        accum_out=out_t[:, 2 * pci:2 * pci + 1])

            def prefetch(ci):
                """Emit chunk ci's loads + products (sq on ACT, prod on DVE).

                Called one chunk ahead so these get higher scheduler priority
                than the previous chunk's ACT tail — keeps both engines fed.
                """
                r, j0, j1 = CHUNKS[ci]
                fc = j1 - j0
                src_yh = yh_d.ap()[r].rearrange(
                    "c (p h) w -> p c (h w)", p=P, h=3)[:, :, j0:j1]
                src_yy = yy_d.ap()[r].rearrange(
                    "c (p h) w -> p c (h w)", p=P, h=3)[:, :, j0:j1]
                yh_t = pin.tile([P, C, fc], F32, name="yh_t", tag="yh")
                nc.sync.dma_start(out=yh_t[:, :, :], in_=src_yh)
                yy_t = pin.tile([P, C, fc], F32, name="yy_t", tag="yy")
                nc.sync.dma_start(out=yy_t[:, :, :], in_=src_yy)
                # PS channels: 0..2 = yh*y (DVE), 3..5 = yh^2 (ACT Square)
                ps = pbig.tile([P, 2 * C, fc], F32, name="ps", tag="ps")
                nc.scalar.activation(ps[:, C:2 * C, :], yh_t[:, :, :], ACT.Square)
                nc.vector.tensor_mul(ps[:, 0:C, :], yh_t[:, :, :], yy_t[:, :, :])
                return ps

            for ci, (r, j0, j1) in enumerate(CHUNKS):
                fc = j1 - j0
                F = fc
                ps = prefetch(ci)

                # ---- paired channel reductions: (x0+x1)+x2 for dot & n2 ----
                # channel pairs {0,3}, {1,4}, {2,5} via stride-3 views
                t1 = psm.tile([P, 2, F], F32, name="t1", tag="t1", bufs=2)
                nc.vector.tensor_add(t1[:, :, :], ps[:, 0:4:3, :], ps[:, 1:5:3, :])
                t2 = psm.tile([P, 2, F], F32, name="t2", tag="t2", bufs=2)
                nc.vector.tensor_add(t2[:, :, :], t1[:, :, :], ps[:, 2:6:3, :])
                dot = t2[:, 0, :]
                n2 = t2[:, 1, :]

                # previous chunk's masked sum fills the DVE's rsq2 wait
                flush_mth()

                # ---- rsq2 = 2/sqrt(n2) via ACT: exp(-0.5*ln(n2) + ln2) -----
                lnn = psm.tile([P, F], F32, name="lnn", tag="lnn", bufs=2)
                nc.scalar.activation(lnn[:, :], n2[:, :], ACT.Ln)
                rsq2 = lnn  # in-place exp over the ln output
                nc.scalar.activation(rsq2[:, :], lnn[:, :], ACT.Exp,
                                     bias=cb_ln2[:, :], scale=-0.5)

                # ---- w = dot * rsq2 = 2*dp ; relu_out = relu((2-umin) - w) -
                w_t = psm.tile([P, F], F32, name="w_t", tag="w_t", bufs=2)
                if W_ON_POOL:
                    nc.gpsimd.tensor_mul(w_t[:, :], dot[:, :], rsq2[:, :])
                else:
                    nc.vector.tensor_mul(w_t[:, :], dot[:, :], rsq2[:, :])
                relu_t = psm.tile([P, F], F32, name="relu_t", tag="relu_t", bufs=2)
                nc.scalar.activation(relu_t[:, :], w_t[:, :], ACT.Relu,
                                     bias=cb_relu[:, :], scale=-1.0)

                # ---- s = sqrt(relu_out + umin) = exp(0.5*ln(u3)) -----------
                lnu = psm.tile([P, F], F32, name="lnu", tag="lnu", bufs=2)
                nc.scalar.activation(lnu[:, :], relu_t[:, :], ACT.Ln,
                                     bias=cb_umin[:, :], scale=1.0)
                s_t = lnu  # in-place exp over the ln output
                nc.scalar.activation(s_t[:, :], lnu[:, :], ACT.Exp,
                                     bias=cb_zero[:, :], scale=0.5)

                if PRECISE_P:
                    p_t = psm.tile([P, F], F32, name="p_t", tag="p_t")
                    nc.scalar.activation(p_t[:, :], relu_t[:, :], ACT.Copy,
                                         bias=P_BIAS, scale=P_SCALE)
                    # maskp = (relu_out < mthresh) * p ; cntp = sum
                    maskp = psm.tile([P, F], F32, name="maskp", tag="maskp")
                    nc.vector.scalar_tensor_tensor(
                        maskp[:, :], relu_t[:, :], MTHRESH, p_t[:, :],
                        ALU.is_lt, ALU.mult,
                        accum_out=out_t[:, 2 * ci + 1:2 * ci + 2])
                    # mth = maskp * s ; sump = sum
                    mth = psm.tile([P, F], F32, name="mth", tag="mth")
                    nc.vector.scalar_tensor_tensor(
                        mth[:, :], maskp[:, :], 1.0, s_t[:, :],
                        ALU.mult, ALU.mult,
                        accum_out=out_t[:, 2 * ci:2 * ci + 1])
                else:
                    # mth = (relu_out < mthresh) * s ; sump = sum  (p dropped)
                    # no explicit count: host bounds count <= sump/theta_min
                    pending.append((relu_t, s_t, ci))

            flush_mth()
            nc.sync.dma_start(out=out_d.ap()[:, :], in_=out_t[:, :])

    nc.compile()
    return nc


def _get_nc():
    if "nc" not in _CACHE:
        _CACHE["nc"] = _build_nc()
    return _CACHE["nc"]


# ---------------- host-side exact fallback (mirrors the reference) ----------
def _host_row_loss(yh_row, yy_row):
    """Exact numpy mirror of the reference for one batch row.

    yh_row, yy_row: [C, H, W] float32.  Returns the row's loss contribution.
    """
    f32 = np.float32
    yh = yh_row.astype(f32)
    yy = yy_row.astype(f32)
    mag = np.sqrt((yh.astype(f32) ** 2).sum(0, dtype=f32), dtype=f32)
    y_norm = (yh / mag).astype(f32)
    dp = (y_norm * yy).sum(0, dtype=f32).astype(f32)
    dpc = np.clip(dp, f32(-1.0 + EPS), f32(1.0 - EPS)).astype(f32)
    theta = np.arccos(dpc).astype(f32).ravel()
    vals = np.sort(theta)
    if vals[PID] < f32(THRESH):
        loss = vals[:PID].sum(dtype=f32)
    else:
        loss = vals[vals < f32(THRESH)].sum(dtype=f32)
    return float(loss)


# ---------------- entry point ------------------------------------------------
def kernel(y_hat, y):
    from concourse.bass_utils import run_bass_kernel_spmd

    y_hat = np.ascontiguousarray(np.asarray(y_hat, dtype=np.float32))
    y = np.ascontiguousarray(np.asarray(y, dtype=np.float32))
    assert y_hat.shape == (B, C, H, W) and y.shape == (B, C, H, W)

    nc = _get_nc()
    in_maps = []
    for i in range(NCORES):
        sl = slice(i * ROWS_PER_CORE, (i + 1) * ROWS_PER_CORE)
        in_maps.append({"yh": y_hat[sl], "yy": y[sl]})

    res = run_bass_kernel_spmd(nc, in_maps, core_ids=list(range(NCORES)))

    total = 0.0
    for i, r_out in enumerate(res.results):
        part = r_out["partials"].astype(np.float64)  # [128, 2*len(CHUNKS)]
        sump_row = np.zeros(ROWS_PER_CORE)
        acc_row = np.zeros(ROWS_PER_CORE)
        for ci, (r, j0, j1) in enumerate(CHUNKS):
            sump_row[r] += part[:, 2 * ci].sum()
            acc_row[r] += part[:, 2 * ci + 1].sum()
        for r in range(ROWS_PER_CORE):
            b = i * ROWS_PER_CORE + r
            if PRECISE_P:
                cnt_hi = acc_row[r] + CNT_SLACK       # p-weighted count >= count
            else:
                # every masked pixel contributes >= theta_min ~ 4.88e-4 to
                # sump, so count <= sump/theta_min (conservative lower theta)
                cnt_hi = sump_row[r] / 4.87e-4 + CNT_SLACK
            if cnt_hi <= PID:
                # cond false for sure: masked-threshold sum (device value)
                total += sump_row[r]
            else:
                # percentile branch possible: recompute this row exactly
                total += _host_row_loss(y_hat[b], y[b])
    return np.float32(total)


if __name__ == "__main__":
    rng = np.random.default_rng(0)
    yh = rng.normal(size=(B, C, H, W)).astype(np.float32)
    yy = rng.normal(size=(B, C, H, W)).astype(np.float32)
    print("kernel loss:", kernel(yh, yy))


# revision 41
# speedup vs baseline: 1.0349x; 1.0349x over previous
"""Trainium2 Bass kernel for nn_EpipolarLoss (epipolar angular loss).

Reference semantics (per batch row b):
    dp     = sum_c(y_hat_c * y_c) / ||y_hat||          [per pixel]
    theta  = acos(clip(dp, -1+eps, 1-eps))
    vals   = sort(theta_row)
    cond_b = vals[pid] < 0.1            (pid = N//2)
    loss_b = sum(pid smallest)          if cond_b
           = sum(theta[theta < 0.1])    otherwise
    loss   = sum_b loss_b

Device strategy (pure data parallel, B=32 over 8 cores, 4 rows/core):
  The sort is avoided: cond_b <=> count(theta < 0.1) > pid.  Each core
  computes, per processing chunk, per SBUF partition, SUMP = sum over
  masked pixels of theta (mask: theta < 0.1).  For masked pixels theta is
  evaluated with the small-angle series acos(x) ~ sqrt(2u), u = 1-x
  (|relative error| <= ~5e-7 on the masked domain), so no acos LUT is
  needed.  Engine split per chunk (ScalarE's Rsqrt/Reciprocal LUTs are
  banned for accuracy, and everything here fits one activation table set,
  natural_log_exp_and_others):
    DVE : prod = yh*y;  paired channel adds -> (dot, n2);  w = dot*rsq2;
          fused masked sum:  mth = (relu_out < mthresh)*s  [+accum]
    ACT : sq = yh^2;  rsq2 = exp(-0.5*ln(n2) + ln2) = 2/sqrt(n2);
          relu_out = relu((2-umin) - w)  [= clamp of u3 = 2-2*dp];
          s = exp(0.5*ln(relu_out + umin)) = sqrt(u3) ~= theta
  GpSimd/PE are intentionally idle: GpSimd shares an SBUF port with DVE
  (measured contention), PE fp32 matmul is 4 cyc/col and fp32r is a
  reduced-precision format.
  The host combines chunk partials, proves cond_b false via
  count <= SUMP/theta_min (theta_min = acos(1-1e-7) ~ 4.88e-4), and falls
  back to an exact numpy sort mirror of the reference for any ambiguous
  row (never taken for randn-scale inputs; also catches NaNs).
"""

import numpy as np

# ---------------- problem constants (hardcoded, self-contained) -------------
B, C, H, W = 32, 3, 384, 512
N = H * W                      # 196608 pixels per row
PID = int(N * 0.5)             # 98304
THRESH = 0.1
EPS = 1e-7
NCORES = 8
ROWS_PER_CORE = B // NCORES    # 4
P = 128                        # SBUF partitions
F = N // P                     # 1536 pixels per partition per row
# processing chunks (row, j0, j1) in per-partition pixel units: row 0 split in
# quarters (fast pipeline ramp-in), row 3 in halves (fast drain)
CHUNKS = [(0, 0, 384), (0, 384, 768), (0, 768, 1152), (0, 1152, 1536),
          (1, 0, 768), (1, 768, 1536), (2, 0, 768), (2, 768, 1536),
          (3, 0, 768), (3, 768, 1536)]

_f32 = np.float32
UMIN = float(_f32(2.0) - _f32(2.0) * (_f32(1.0) - _f32(EPS)))   # 2-2*(1-eps), exact
C01 = float(_f32(np.cos(THRESH)))                               # cos(0.1) in fp32
UMASK = float(_f32(2.0) - _f32(2.0) * _f32(C01))
MTHRESH = float(_f32(UMASK) - _f32(UMIN))        # mask: relu_out < MTHRESH
RELU_BIAS = float(_f32(2.0) - _f32(UMIN))        # relu(u + (2-umin))
P_SCALE = 1.0 / 24.0                             # p = 1 + u3/24  (u3 = 2u)
P_BIAS = float(_f32(1.0) + _f32(UMIN / 24.0))
# p upper bound on masked pixels (for exact-count recovery bounds)
P_MAX = 1.0 + UMASK / 24.0 + 1e-6

# count ambiguity slack (mask boundary flips are ~O(1) pixels; be generous)
CNT_SLACK = 4096.0

# perf/accuracy knobs
PRECISE_P = False   # True: keep p=1+u3/24 factor + p-weighted count (extra DVE op)
PROD_ON_POOL = False  # GpSimd shares an SBUF port with DVE: contention loses
W_ON_POOL = False  # Pool w-mult measured worse (port contention + chain latency)
LN2 = float(np.log(2.0))

# ---------------- device kernel ---------------------------------------------
_CACHE = {}


def _patch_act_tables():
    """Force every activation onto the natural_log_exp_and_others table set.

    bacc's set chooser otherwise flip-flops between sets (Ln -> natural_log,
    Exp -> exp_and_others, ...) costing ~1.3us per ACT_TABLE_LOAD.  All
    functions used here (Square/Ln/Exp/Relu/Sign/Copy) live in one set, so
    blank out every other set (keeping dict order => act_func_set_id stays
    aligned with act_info.json).
    """
    import concourse.bacc as bacc_mod
    import concourse.hw_specs as hw_specs

    if getattr(bacc_mod, "_epi_act_patch", False):
        return
    orig = hw_specs.get_activation_tables

    def patched(arch):
        t = orig(arch)
        keep = "natural_log_exp_and_others"
        assert keep in t
        return {k: (v if k == keep else set()) for k, v in t.items()}

    bacc_mod.get_activation_tables = patched
    bacc_mod._epi_act_patch = True


def _build_nc():
    import concourse.bass as bass  # noqa: F401  (AP types)
    import concourse.tile as tile
    from concourse import bacc, mybir

    _patch_act_tables()

    F32 = mybir.dt.float32
    ALU = mybir.AluOpType
    ACT = mybir.ActivationFunctionType

    nc = bacc.Bacc(dynamic_dma_scratch_size=4096)
    yh_d = nc.dram_tensor("yh", [ROWS_PER_CORE, C, H, W], F32, kind="ExternalInput")
    yy_d = nc.dram_tensor("yy", [ROWS_PER_CORE, C, H, W], F32, kind="ExternalInput")
    out_d = nc.dram_tensor("partials", [P, 2 * len(CHUNKS)], F32,
                           kind="ExternalOutput")

    with tile.TileContext(nc, pool_alloc_mode="queue") as tc:
        with (
            tc.tile_pool(name="pin", bufs=2) as pin,
            tc.tile_pool(name="pbig", bufs=1) as pbig,
            tc.tile_pool(name="psm", bufs=1) as psm,
            tc.tile_pool(name="pconst", bufs=1) as pconst,
        ):
            cb_relu = pconst.tile([P, 1], F32, name="cb_relu", tag="cb_relu")
            nc.vector.memset(cb_relu[:, :], RELU_BIAS)
            cb_umin = pconst.tile([P, 1], F32, name="cb_umin", tag="cb_umin")
            nc.vector.memset(cb_umin[:, :], UMIN)
            cb_mth = pconst.tile([P, 1], F32, name="cb_mth", tag="cb_mth")
            nc.vector.memset(cb_mth[:, :], MTHRESH)
            cb_ln2 = pconst.tile([P, 1], F32, name="cb_ln2", tag="cb_ln2")
            nc.vector.memset(cb_ln2[:, :], LN2)
            cb_zero = pconst.tile([P, 1], F32, name="cb_zero", tag="cb_zero")
            nc.vector.memset(cb_zero[:, :], 0.0)
            out_t = pconst.tile([P, 2 * len(CHUNKS)], F32, name="out_t",
                                tag="out_t")

            # software-pipelined mth: chunk ci's masked-sum STT is emitted
            # during chunk ci+1 (right after its t2 add), so the DVE never
            # stalls waiting for the 3-op ACT chain that produces s_ci.
            pending = []  # (relu_t, s_t, ci)

            def flush_mth():
                while pending:
                    prelu, ps_t, pci = pending.pop(0)
                    mth = psm.tile([P, prelu.shape[-1]], F32, name="mth",
                                   tag="mth", bufs=2)
                    nc.vector.scalar_tensor_tensor(
                        mth[:, :], prelu[:, :], MTHRESH, ps_t[:, :],
                        ALU.is_lt, ALU.mult,
                        accum_out=out_t[:, 2 * pci:2 * pci + 1])

            def prefetch(ci):
                """Emit chunk ci's loads + products (sq on ACT, prod on DVE).

                Called one chunk ahead so these get higher scheduler priority
                than the previous chunk's ACT tail — keeps both engines fed.
                """
                r, j0, j1 = CHUNKS[ci]
                fc = j1 - j0
                src_yh = yh_d.ap()[r].rearrange(
                    "c (p h) w -> p c (h w)", p=P, h=3)[:, :, j0:j1]
                src_yy = yy_d.ap()[r].rearrange(
                    "c (p h) w -> p c (h w)", p=P, h=3)[:, :, j0:j1]
                yh_t = pin.tile([P, C, fc], F32, name="yh_t", tag="yh")
                nc.sync.dma_start(out=yh_t[:, :, :], in_=src_yh)
                yy_t = pin.tile([P, C, fc], F32, name="yy_t", tag="yy")
                nc.sync.dma_start(out=yy_t[:, :, :], in_=src_yy)
                # PS channels: 0..2 = yh*y (DVE), 3..5 = yh^2 (ACT Square)
                ps = pbig.tile([P, 2 * C, fc], F32, name="ps", tag="ps")
                nc.scalar.activation(ps[:, C:2 * C, :], yh_t[:, :, :], ACT.Square)
                nc.vector.tensor_mul(ps[:, 0:C, :], yh_t[:, :, :], yy_t[:, :, :])
                return ps

            for ci, (r, j0, j1) in enumerate(CHUNKS):
                fc = j1 - j0
                F = fc
                ps = prefetch(ci)

                # ---- paired channel reductions: (x0+x1)+x2 for dot & n2 ----
                # channel pairs {0,3}, {1,4}, {2,5} via stride-3 views
                t1 = psm.tile([P, 2, F], F32, name="t1", tag="t1", bufs=2)
                nc.vector.tensor_add(t1[:, :, :], ps[:, 0:4:3, :], ps[:, 1:5:3, :])
                t2 = psm.tile([P, 2, F], F32, name="t2", tag="t2", bufs=2)
                nc.vector.tensor_add(t2[:, :, :], t1[:, :, :], ps[:, 2:6:3, :])
                dot = t2[:, 0, :]
                n2 = t2[:, 1, :]

                # previous chunk's masked sum fills the DVE's rsq2 wait
                flush_mth()

                # ---- rsq2 = 2/sqrt(n2) via ACT: exp(-0.5*ln(n2) + ln2) -----
                lnn = psm.tile([P, F], F32, name="lnn", tag="lnn", bufs=2)
                nc.scalar.activation(lnn[:, :], n2[:, :], ACT.Ln)
                rsq2 = lnn  # in-place exp over the ln output
                nc.scalar.activation(rsq2[:, :], lnn[:, :], ACT.Exp,
                                     bias=cb_ln2[:, :], scale=-0.5)

                # ---- w = dot * rsq2 = 2*dp ; relu_out = relu((2-umin) - w) -
                w_t = psm.tile([P, F], F32, name="w_t", tag="w_t", bufs=2)
                if W_ON_POOL:
                    nc.gpsimd.tensor_mul(w_t[:, :], dot[:, :], rsq2[:, :])
                else:
                    nc.vector.tensor_mul(w_t[:, :], dot[:, :], rsq2[:, :])
                relu_t = psm.tile([P, F], F32, name="relu_t", tag="relu_t", bufs=2)
                nc.scalar.activation(relu_t[:, :], w_t[:, :], ACT.Relu,
                                     bias=cb_relu[:, :], scale=-1.0)

                # ---- s = sqrt(relu_out + umin) = exp(0.5*ln(u3)) -----------
                lnu = psm.tile([P, F], F32, name="lnu", tag="lnu", bufs=2)
                nc.scalar.activation(lnu[:, :], relu_t[:, :], ACT.Ln,
                                     bias=cb_umin[:, :], scale=1.0)
                s_t = lnu  # in-place exp over the ln output
                nc.scalar.activation(s_t[:, :], lnu[:, :], ACT.Exp,
                                     bias=cb_zero[:, :], scale=0.5)

                if PRECISE_P:
                    p_t = psm.tile([P, F], F32, name="p_t", tag="p_t")
                    nc.scalar.activation(p_t[:, :], relu_t[:, :], ACT.Copy,
                                         bias=P_BIAS, scale=P_SCALE)
                    # maskp = (relu_out < mthresh) * p ; cntp = sum
                    maskp = psm.tile([P, F], F32, name="maskp", tag="maskp")
                    nc.vector.scalar_tensor_tensor(
                        maskp[:, :], relu_t[:, :], MTHRESH, p_t[:, :],
                        ALU.is_lt, ALU.mult,
                        accum_out=out_t[:, 2 * ci + 1:2 * ci + 2])
                    # mth = maskp * s ; sump = sum
                    mth = psm.tile([P, F], F32, name="mth", tag="mth")
                    nc.vector.scalar_tensor_tensor(
                        mth[:, :], maskp[:, :], 1.0, s_t[:, :],
                        ALU.mult, ALU.mult,
                        accum_out=out_t[:, 2 * ci:2 * ci + 1])
                else:
                    # mth = (relu_out < mthresh) * s ; sump = sum  (p dropped)
                    # no explicit count: host bounds count <= sump/theta_min
                    pending.append((relu_t, s_t, ci))

            flush_mth()
            nc.sync.dma_start(out=out_d.ap()[:, :], in_=out_t[:, :])

    nc.compile()
    return nc


def _get_nc():
    if "nc" not in _CACHE:
        _CACHE["nc"] = _build_nc()
    return _CACHE["nc"]


# ---------------- host-side exact fallback (mirrors the reference) ----------
def _host_row_loss(yh_row, yy_row):
    """Exact numpy mirror of the reference for one batch row.

    yh_row, yy_row: [C, H, W] float32.  Returns the row's loss contribution.
    """
    f32 = np.float32
    yh = yh_row.astype(f32)
    yy = yy_row.astype(f32)
    mag = np.sqrt((yh.astype(f32) ** 2).sum(0, dtype=f32), dtype=f32)
    y_norm = (yh / mag).astype(f32)
    dp = (y_norm * yy).sum(0, dtype=f32).astype(f32)
    dpc = np.clip(dp, f32(-1.0 + EPS), f32(1.0 - EPS)).astype(f32)
    theta = np.arccos(dpc).astype(f32).ravel()
    vals = np.sort(theta)
    if vals[PID] < f32(THRESH):
        loss = vals[:PID].sum(dtype=f32)
    else:
        loss = vals[vals < f32(THRESH)].sum(dtype=f32)
    return float(loss)


# ---------------- entry point ------------------------------------------------
def kernel(y_hat, y):
    from concourse.bass_utils import run_bass_kernel_spmd

    y_hat = np.ascontiguousarray(np.asarray(y_hat, dtype=np.float32))
    y = np.ascontiguousarray(np.asarray(y, dtype=np.float32))
    assert y_hat.shape == (B, C, H, W) and y.shape == (B, C, H, W)

    nc = _get_nc()
    in_maps = []
    for i in range(NCORES):
        sl = slice(i * ROWS_PER_CORE, (i + 1) * ROWS_PER_CORE)
        in_maps.append({"yh": y_hat[sl], "yy": y[sl]})

    res = run_bass_kernel_spmd(nc, in_maps, core_ids=list(range(NCORES)))

    total = 0.0
    for i, r_out in enumerate(res.results):
        part = r_out["partials"].astype(np.float64)  # [128, 2*len(CHUNKS)]
        sump_row = np.zeros(ROWS_PER_CORE)
        acc_row = np.zeros(ROWS_PER_CORE)
        for ci, (r, j0, j1) in enumerate(CHUNKS):
            sump_row[r] += part[:, 2 * ci].sum()
            acc_row[r] += part[:, 2 * ci + 1].sum()
        for r in range(ROWS_PER_CORE):
            b = i * ROWS_PER_CORE + r
            if PRECISE_P:
                cnt_hi = acc_row[r] + CNT_SLACK       # p-weighted count >= count
            else:
                # every masked pixel contributes >= theta_min ~ 4.88e-4 to
                # sump, so count <= sump/theta_min (conservative lower theta)
                cnt_hi = sump_row[r] / 4.87e-4 + CNT_SLACK
            if cnt_hi <= PID:
                # cond false for sure: masked-threshold sum (device value)
                total += sump_row[r]
            else:
                # percentile branch possible: recompute this row exactly
                total += _host_row_loss(y_hat[b], y[b])
    return np.float32(total)


if __name__ == "__main__":
    rng = np.random.default_rng(0)
    yh = rng.normal(size=(B, C, H, W)).astype(np.float32)
    yy = rng.normal(size=(B, C, H, W)).astype(np.float32)
    print("kernel loss:", kernel(yh, yy))
